# revision 11
# baseline (speedup 1.0000x reference)
"""Bass/Tile Trainium2 kernel for the additive-attention module (v3).

reference (per batch row b):
    q = hidden_state @ Wa.T + ba                 # [A]
    k = feature_vectors[b] @ Ua.T                # [L, A]
    e = tanh(q + k) @ w                          # [L]
    attn = softmax(e)                            # [L]
    context[b] = attn @ feature_vectors[b]       # [M]

Sharding: data-parallel over batch B=64 -> 8 cores x 8 rows, params
replicated, no collectives.

v3 changes vs v2:
  - fvT row loads moved from the gpsimd software DGE to the two
    hardware DGE rings (sync + scalar engines), alternating per row
  - gpsimd removed from the per-row path entirely: cross-partition max
    is PE-transpose + DVE free-max + PE ones-matmul broadcast; the
    softmax normalizer Z is a PE ones-matmul partition sum
  - weighted sum is one fused tensor_tensor_reduce per m-half
  - p broadcast [1,L] -> [128,L] via an SBUF->SBUF DMA with a
    stride-0 partition source AP (no DRAM bounce, no extra HBM reads)
  - per-row context stashed in SBUF, transposed once at the end and
    written with a single well-shaped DMA
  - 3-deep software pipeline: row b computes k/tanh/e while row b-1
    runs softmax/p-chain and row b-2 runs the weighted sum
"""

import numpy as np

B, R, M, A, L = 64, 512, 256, 256, 4096
NCORES = 8
BLOC = B // NCORES  # 8 batch rows per core
NJG = 4  # j-groups of 1024 l-columns
JGW = L // NJG  # 1024
JW2 = JGW // 2  # 512 (psum bank width in f32)
NL = L // 128  # 32 e-columns

_CACHE = {}


def _build():
    from contextlib import ExitStack

    import concourse.bacc as bacc
    import concourse.bass as bass
    import concourse.mybir as mybir
    import concourse.tile as tile
    from concourse.masks import make_identity

    f32 = mybir.dt.float32
    f16 = mybir.dt.float16
    AF = mybir.ActivationFunctionType
    ALU = mybir.AluOpType

    nc = bacc.Bacc("TRN2", target_bir_lowering=False, debug=False,
                   num_devices=NCORES)

    hs = nc.dram_tensor("hidden_state", [BLOC, R], f32, kind="ExternalInput").ap()
    fvt_d = nc.dram_tensor("fvT", [BLOC, M, L], f16, kind="ExternalInput").ap()
    Wa = nc.dram_tensor("Wa", [A, R], f32, kind="ExternalInput").ap()
    Ua = nc.dram_tensor("Ua", [A, M], f32, kind="ExternalInput").ap()
    w = nc.dram_tensor("w", [A, 1], f32, kind="ExternalInput").ap()
    ba = nc.dram_tensor("ba", [1, A], f32, kind="ExternalInput").ap()
    ctx_out = nc.dram_tensor("context", [BLOC, M], f32, kind="ExternalOutput").ap()

    with tile.TileContext(nc) as tc, ExitStack() as ctx:
        singles = ctx.enter_context(tc.tile_pool(name="singles", bufs=1))
        ldpool = ctx.enter_context(tc.tile_pool(name="ldpool", bufs=2))
        fvpool = ctx.enter_context(tc.tile_pool(name="fvpool", bufs=4))
        tpool = ctx.enter_context(tc.tile_pool(name="tpool", bufs=2))
        bcpool = ctx.enter_context(tc.tile_pool(name="bcpool", bufs=3))
        trashp = ctx.enter_context(tc.tile_pool(name="trashp", bufs=2))
        small = ctx.enter_context(tc.tile_pool(name="small", bufs=4))
        ps_k = ctx.enter_context(tc.tile_pool(name="ps_k", bufs=2, space="PSUM"))
        ps_e = ctx.enter_context(tc.tile_pool(name="ps_e", bufs=2, space="PSUM"))
        ps_sm = ctx.enter_context(tc.tile_pool(name="ps_sm", bufs=2, space="PSUM"))

        # ---- row-0/1 fvT loads first so the PE can start ASAP ----
        def fvt_load(b, fvt):
            # 2MB per row on a hardware DGE ring, alternating rings per
            # row; split in two L-halves so jg0/1 can start early.
            eng = nc.sync  # fvt loads own the sync ring exclusively
            for half in range(2):
                src = bass.AP(tensor=fvt_d.tensor,
                              offset=b * M * L + half * (L // 2),
                              ap=[[L, 128], [128 * L, 2], [1, L // 2]])
                eng.dma_start(
                    out=fvt[:, :, half * (L // 2):(half + 1) * (L // 2)],
                    in_=src)

        fvt_tiles = {}
        for b in range(2):
            fvt_tiles[b] = fvpool.tile([128, 2, L], f16, tag="fvt", name="fvt")
            fvt_load(b, fvt_tiles[b])

        ident = singles.tile([128, 128], f32, tag="ident", name="ident")
        make_identity(nc, ident)
        ident16 = singles.tile([128, 128], f16, tag="ident16", name="ident16")
        make_identity(nc, ident16)
        # ones tiles for partition broadcast / partition sum matmuls
        ones32 = singles.tile([128, 128], f32, tag="ones32", name="ones32")
        nc.vector.memset(ones32, 1.0)

        # ---- parameters into contraction-major layouts ----
        # WaT[rt] [128(r), 256(a)] fp32: WaT[rt][k, a] = Wa[a, 128*rt + k]
        WaT = [singles.tile([128, A], f32, tag=f"WaT{rt}", name=f"WaT{rt}")
               for rt in range(4)]
        for at in range(2):
            wa_nat = ldpool.tile([128, R], f32, tag="ld", name="ld")
            nc.sync.dma_start(out=wa_nat, in_=Wa[at * 128:(at + 1) * 128, :])
            for rt in range(4):
                ps = ps_sm.tile([128, 128], f32, tag="sm", name="sm")
                nc.tensor.transpose(ps, wa_nat[:, rt * 128:(rt + 1) * 128], ident)
                nc.vector.tensor_copy(out=WaT[rt][:, at * 128:(at + 1) * 128],
                                      in_=ps)
        # UaT[mh] [128(m), 256(a)] fp16: UaT[mh][k, a] = Ua[a, 128*mh + k]
        UaT = [singles.tile([128, A], f16, tag=f"UaT{mh}", name=f"UaT{mh}")
               for mh in range(2)]
        for at in range(2):
            ua_nat = ldpool.tile([128, M], f32, tag="ld", name="ld")
            nc.sync.dma_start(out=ua_nat, in_=Ua[at * 128:(at + 1) * 128, :])
            for mh in range(2):
                ps = ps_sm.tile([128, 128], f32, tag="sm", name="sm")
                nc.tensor.transpose(ps, ua_nat[:, mh * 128:(mh + 1) * 128], ident)
                nc.vector.tensor_copy(out=UaT[mh][:, at * 128:(at + 1) * 128],
                                      in_=ps)
        # w as fp16 stationary columns [128, 1] per a-half (cast during DMA)
        w_sb = [singles.tile([128, 1], f16, tag=f"w{ah}", name=f"w{ah}")
                for ah in range(2)]
        for ah in range(2):
            nc.gpsimd.dma_start(out=w_sb[ah], in_=w[ah * 128:(ah + 1) * 128, :])

        # hsT[rt] [128(r), BLOC] fp32
        hsT = [singles.tile([128, BLOC], f32, tag=f"hsT{rt}", name=f"hsT{rt}")
               for rt in range(4)]
        for rt in range(4):
            src = bass.AP(tensor=hs.tensor, offset=rt * 128,
                          ap=[[1, 128], [R, BLOC]])
            nc.sync.dma_start(out=hsT[rt], in_=src)

        # q = hs @ Wa.T + ba   -> [BLOC, A] fp32
        q_ps = ps_sm.tile([BLOC, A], f32, tag="sm", name="sm")
        for rt in range(4):
            nc.tensor.matmul(q_ps, lhsT=hsT[rt], rhs=WaT[rt],
                             start=(rt == 0), stop=(rt == 3))
        ba_b = singles.tile([BLOC, A], f32, tag="ba", name="ba")
        nc.sync.dma_start(out=ba_b,
                          in_=bass.AP(tensor=ba.tensor, offset=0,
                                      ap=[[0, BLOC], [1, A]]))
        q_sb = singles.tile([BLOC, A], f32, tag="q", name="q")
        nc.vector.tensor_add(q_sb, q_ps, ba_b)
        # qT[ah] [128(a), BLOC] fp32
        qT = [singles.tile([128, BLOC], f32, tag=f"qT{ah}", name=f"qT{ah}")
              for ah in range(2)]
        for ah in range(2):
            ps = ps_sm.tile([128, BLOC], f32, tag="sm", name="sm")
            nc.tensor.transpose(ps, q_sb[:, ah * 128:(ah + 1) * 128],
                                ident[:BLOC, :BLOC])
            nc.vector.tensor_copy(out=qT[ah], in_=ps)

        # per-row context stash [128(m%128), 8(b), 2(mh)] f32
        ctx_all = singles.tile([128, BLOC, 2], f32, tag="ctxall", name="ctxall")

        # ---- per-batch-row pipeline ----
        state = {}  # b -> dict(fvt, psE, t, p_t, ...)

        def emit_load(b):
            if b in fvt_tiles:
                st_fvt = fvt_tiles[b]
            else:
                st_fvt = fvpool.tile([128, 2, L], f16, tag="fvt", name="fvt")
                fvt_load(b, st_fvt)
                fvt_tiles[b] = st_fvt

        def emit_start_row(b):
            st = {"fvt": fvt_tiles.pop(b)}
            st["psE"] = ps_e.tile([128, NL], f32, tag="psE", name="psE")
            st["t"] = tpool.tile([128, 2, L], f16, tag="t", name="t")
            state[b] = st

        def emit_k(b, jg):
            # k chunk [a, l] for both a-halves: psk[ah][:, c, :]
            st = state[b]
            psks = []
            for ah in range(2):
                psk = ps_k.tile([128, 2, JW2], f32, tag="psk", name="psk")
                for mh in range(2):
                    lhsT = UaT[mh][:, ah * 128:(ah + 1) * 128]
                    for c in range(2):
                        lo = jg * JGW + c * JW2
                        nc.tensor.matmul(
                            psk[:, c, :],
                            lhsT=lhsT,
                            rhs=st["fvt"][:, mh, lo:lo + JW2],
                            start=(mh == 0), stop=(mh == 1))
                psks.append(psk)
            st[("psk", jg)] = psks

        def emit_tanh(b, jg):
            st = state[b]
            psks = st.pop(("psk", jg))
            for ah in range(2):
                nc.scalar.activation(
                    out=st["t"][:, ah, jg * JGW:(jg + 1) * JGW],
                    in_=psks[ah], func=AF.Tanh,
                    bias=qT[ah][:, b:b + 1], scale=1.0)

        def emit_e(b, jg):
            st = state[b]
            psE = st["psE"]
            for c8 in range(8):
                col = jg * 8 + c8
                for ah in range(2):
                    nc.tensor.matmul(
                        psE[:, col:col + 1],
                        lhsT=st["t"][:, ah, col * 128:(col + 1) * 128],
                        rhs=w_sb[ah],
                        start=(ah == 0), stop=(ah == 1))

        # --- softmax stage 1: cross-partition max without gpsimd ---
        def emit_sm_rowmax(b):
            st = state[b]
            mrow = small.tile([128, 1], f32, tag="mrow", name="mrow")
            nc.vector.reduce_max(out=mrow, in_=st["psE"],
                                 axis=mybir.AxisListType.X)
            st["mrow"] = mrow

        def emit_sm_maxT(b):
            st = state[b]
            mT = ps_sm.tile([1, 128], f32, tag="sm", name="sm")
            nc.tensor.transpose(mT, st["mrow"], ident)
            st["mT"] = mT

        def emit_sm_max128(b):
            st = state[b]
            mall = small.tile([1, 1], f32, tag="mall", name="mall")
            nc.vector.reduce_max(out=mall, in_=st.pop("mT"),
                                 axis=mybir.AxisListType.X)
            st["mall"] = mall

        def emit_sm_maxbcast(b):
            st = state[b]
            mbc = ps_sm.tile([128, 1], f32, tag="sm", name="sm")
            nc.tensor.matmul(mbc, lhsT=ones32[:1, :], rhs=st.pop("mall"),
                             start=True, stop=True)
            st["mbc"] = mbc

        def emit_sm_negm(b):
            st = state[b]
            negm = small.tile([128, 1], f32, tag="negm", name="negm")
            nc.vector.tensor_scalar_mul(negm, st.pop("mbc"), -1.0)
            st["negm"] = negm

        # --- softmax stage 2: exp, Z, p transpose + broadcast ---
        def emit_sm_exp(b):
            st = state[b]
            p_t = small.tile([128, NL], f16, tag="p_t", name="p_t")
            srow = small.tile([128, 1], f32, tag="srow", name="srow")
            nc.scalar.activation(out=p_t, in_=st.pop("psE"), func=AF.Exp,
                                 bias=st.pop("negm"), scale=1.0,
                                 accum_out=srow)
            st["p_t"] = p_t
            st["srow"] = srow

        def emit_sm_Z(b):
            st = state[b]
            zbc = ps_sm.tile([128, 1], f32, tag="sm", name="sm")
            nc.tensor.matmul(zbc, lhsT=ones32, rhs=st.pop("srow"),
                             start=True, stop=True)
            rz = small.tile([128, 1], f32, tag="rz", name="rz")
            nc.vector.reciprocal(out=rz, in_=zbc)
            st["rz"] = rz

        def emit_sm_pT(b):
            st = state[b]
            pT_ps = ps_sm.tile([32, 128], f16, tag="sm", name="sm")
            nc.tensor.transpose(pT_ps, st.pop("p_t"), ident16)
            pT_sb = small.tile([32, 128], f16, tag="ptsb", name="ptsb")
            nc.vector.tensor_copy(out=pT_sb, in_=pT_ps)
            st["pT_sb"] = pT_sb

        def emit_sm_pbc(b):
            # [32,128] -> [1,4096] linearize into p_bc[0], then a
            # log-doubling partition broadcast to [128, 4096]; all
            # SBUF->SBUF on HW DGE rings (no HBM traffic)
            st = state[b]
            eng = nc.scalar  # p-chains own the scalar ring exclusively
            p_bc = bcpool.tile([128, L], f16, tag="p_bc", name="p_bc")
            eng.dma_start(out=p_bc[0:1, :], in_=st.pop("pT_sb"))
            for k in range(7):
                n = 1 << k
                eng.dma_start(out=p_bc[n:2 * n, :], in_=p_bc[0:n, :])
            st["p_bc"] = p_bc

        def emit_ws(b):
            st = state.pop(b)
            for mh in range(2):
                trash = trashp.tile([128, L], f16, tag="trash", name="trash")
                # NOTE: tensor_tensor_reduce crashes the NEFF on this hw;
                # affine_mul_reduce is the working fused mul+reduce. 1/Z is
                # folded into the affine scale so the accumulator IS the
                # final context value.
                nc.vector.affine_mul_reduce(
                    out=trash, accum_out=ctx_all[:, b, mh:mh + 1],
                    in0=st["p_bc"], in1=st["fvt"][:, mh, :],
                    scale=st["rz"], bias=0.0)

        # ---------------- pipeline ----------------
        for b in range(BLOC):
            emit_start_row(b)
            emit_k(b, 0)
            if b >= 1:
                emit_sm_rowmax(b - 1)
            emit_k(b, 1)
            emit_tanh(b, 0)
            if b >= 1:
                emit_sm_maxT(b - 1)
                emit_sm_max128(b - 1)
            emit_k(b, 2)
            emit_tanh(b, 1)
            if b >= 1:
                emit_sm_maxbcast(b - 1)
                emit_sm_negm(b - 1)
                emit_sm_exp(b - 1)
            emit_e(b, 0)
            if b >= 1:
                emit_sm_Z(b - 1)
            emit_k(b, 3)
            emit_tanh(b, 2)
            emit_e(b, 1)
            if b >= 1:
                emit_sm_pT(b - 1)
                emit_sm_pbc(b - 1)
            emit_tanh(b, 3)
            emit_e(b, 2)
            if b >= 2:
                emit_ws(b - 2)
            if b + 2 < BLOC:
                emit_load(b + 2)
            emit_e(b, 3)

        # tail: softmax + ws for the last two rows
        bl = BLOC - 1
        emit_sm_rowmax(bl)
        emit_sm_maxT(bl)
        emit_sm_max128(bl)
        emit_sm_maxbcast(bl)
        emit_sm_negm(bl)
        emit_sm_exp(bl)
        emit_sm_Z(bl)
        emit_sm_pT(bl)
        emit_sm_pbc(bl)
        emit_ws(bl - 1)
        emit_ws(bl)

        # final output: transpose ctx stash and write one clean DMA
        ctxT = ps_sm.tile([2 * BLOC, 128], f32, tag="sm", name="sm")
        nc.tensor.transpose(ctxT, ctx_all, ident)
        ctxT_sb = small.tile([2 * BLOC, 128], f32, tag="ctxT", name="ctxT")
        nc.vector.tensor_copy(out=ctxT_sb, in_=ctxT)
        # partition j = b*2 + mh  ->  dram offset b*M + mh*128 = j*128
        nc.sync.dma_start(
            out=bass.AP(tensor=ctx_out.tensor, offset=0,
                        ap=[[128, 2 * BLOC], [1, 128]]),
            in_=ctxT_sb)

    nc.compile()
    return nc


def _get_nc():
    if "nc" not in _CACHE:
        _CACHE["nc"] = _build()
    return _CACHE["nc"]


def make_in_maps(inputs):
    """Per-core input dicts for run_bass_kernel_spmd (host-side shard +
    fp16 pre-transpose of feature_vectors)."""
    fv = np.asarray(inputs["feature_vectors"])
    fvT = fv.transpose(0, 2, 1).astype(np.float16)  # [B, M, L] fp16
    hs = np.ascontiguousarray(np.asarray(inputs["hidden_state"]),
                              dtype=np.float32)
    params = {
        "Wa": np.ascontiguousarray(np.asarray(inputs["Wa"]), dtype=np.float32),
        "Ua": np.ascontiguousarray(np.asarray(inputs["Ua"]), dtype=np.float32),
        "w": np.ascontiguousarray(np.asarray(inputs["w"]), dtype=np.float32),
        "ba": np.ascontiguousarray(np.asarray(inputs["ba"]), dtype=np.float32),
    }
    return [
        {
            "hidden_state": hs[c * BLOC:(c + 1) * BLOC],
            "fvT": np.ascontiguousarray(fvT[c * BLOC:(c + 1) * BLOC]),
            **params,
        }
        for c in range(NCORES)
    ]


def kernel(hidden_state, feature_vectors, Wa, Ua, w, ba):
    from concourse.bass_utils import run_bass_kernel_spmd

    nc = _get_nc()
    in_maps = make_in_maps({
        "hidden_state": hidden_state,
        "feature_vectors": feature_vectors,
        "Wa": Wa, "Ua": Ua, "w": w, "ba": ba,
    })
    res = run_bass_kernel_spmd(nc, in_maps, list(range(NCORES)))
    return np.concatenate([res.results[c]["context"] for c in range(NCORES)],
                          axis=0)
